# revision 22
# baseline (speedup 1.0000x reference)
"""GCN message-passing kernel for 8 trn2 NeuronCores.

Math (per reference): h = relu(a @ (x @ W1) + b1); out = h @ W2 + b2
Shapes: x [8,4096,240], a [4096,4096], W1 [240,32], W2 [32,240].

Sharding: 2x4 grid. Core c -> batch group g=c//4 (4 batches), output-row
group j=c%4 (1024 rows). W1/W2 fp16; x AND a host-converted to fp8-e3m4.
PSUM fp32; output fp16.

v3 design (baseline 71.9us -> v2 70.8 -> this):
- HAM warmup: 10 dummy matmuls on a zeroed tile during the NEFF preamble
  so the PE runs at 2.4GHz when real data lands.
- Col-tiled phase 1: 4 concurrent matmuls per (fh, nl) round via
  tile_position (one batch per 32-col strip).
- 8 sub-groups of 512 nodes; PE emission P1(s), P2A(s-1), T(s) keeps
  independent streams mixed (covers DVE handoffs and LDW latency).
- Phase 2 split into mc-major sweeps: sweep A accumulates pa[0] (rows
  0-511) inside the main loop; sweep B (pa[1]) runs after, interleaved
  with phase 3 of mc0 -- half of phase 3 hides under sweep B's matmuls.
  Same total LDW count (bass reloads weights per matmul anyway).
- Transposed phase 3: out[l-partitions, nodes] via 4 row-tiled concurrent
  matmuls (stationary W2 [32,120] per batch at tile_position (32b,0)), so
  b2 is a per-partition bias: one-op cast+bias on vector
  (scalar_tensor_tensor add/bypass) or scalar (Identity ACT with bias AP),
  split 2+2 per lh-block. relu+b1 via vector add/max against a zero tile
  (no Relu table -> no ACT table switches).
- DMA: sync ring = x stream (8 x 491KB) + outputs (4 x 491KB);
  scalar ring = W1/identity/b1 then aT stream (8 x 524KB);
  gpsimd (SWDGE) = small phase-3 consts only.
"""

import sys

if "/opt/trn_rl_repo" not in sys.path:
    sys.path.insert(0, "/opt/trn_rl_repo")

import numpy as np

B, N, F, H, L = 8, 4096, 240, 32, 240
NB = 4        # batches per core
NRC = 1024    # output rows per core
NS = 8        # node sub-groups per core (512 nodes each)
NWARM = 10
TRACE = False

_cache = {}
last_exec_time_ns = None
last_profile_json = None


def _install_ntff_hook():
    import types

    import antenv

    if "antenv.axon_hooks" in sys.modules:
        return
    mod = types.ModuleType("antenv.axon_hooks")
    _state = {"hook": None}
    mod.set_axon_ntff_profile_hook = lambda h: _state.__setitem__("hook", h)
    mod.get_axon_ntff_profile_hook = lambda: _state["hook"]
    sys.modules["antenv.axon_hooks"] = mod
    antenv.axon_hooks = mod
    from trn_agent_boot.trn_boot import _ntff_profile_via_ctypes

    mod.set_axon_ntff_profile_hook(
        _ntff_profile_via_ctypes("/opt/axon/libaxon_pjrt.so")
    )


def _build():
    import concourse.bass as bass
    import concourse.tile as tile
    from concourse import bacc, mybir

    f32 = mybir.dt.float32
    f16 = mybir.dt.float16
    f8 = mybir.dt.float8e3
    ts, ds = bass.ts, bass.ds
    add = mybir.AluOpType.add
    byp = mybir.AluOpType.bypass
    ident = mybir.ActivationFunctionType.Identity
    relu = mybir.ActivationFunctionType.Relu

    nc = bacc.Bacc("TRN2", target_bir_lowering=False, debug=False, num_devices=8)
    # xnb block s: cols [0:4096] x of sub-group s (rows 0:120, layout
    # b*1024 + fh*512 + nl), cols [4096:6144] the mc0 half of the same
    # sub-group's aT block (512*q + c) -- one DMA per loop cycle.
    xnb = nc.dram_tensor("xnb", [NS * 128, 6144], f8, kind="ExternalInput").ap()
    # athd pair t: cols [0:2048]/[2048:4096] = mc1 halves of sub-groups
    # 2t / 2t+1 (512*q + c each)
    athd = nc.dram_tensor("athd", [512, 4096], f8, kind="ExternalInput").ap()
    # packed consts, all f16: cols [0:64] W1 (two fh halves), [64:192]
    # identity, [192:432] W2 tiled 4x vertically, [432] b1 (4x tiled),
    # [433:435] b2 halves
    cpkd = nc.dram_tensor("cpkd", [128, 436], f16, kind="ExternalInput").ap()
    # outp[p, ((mc*2+lh)*4 + b)*512 + c] = out[4g+b, 1024j + 512mc + c, 120lh + p]
    outp = nc.dram_tensor("outp", [120, 8192], f16, kind="ExternalOutput").ap()

    with tile.TileContext(nc) as tc:
        with tc.tile_pool(name="const", bufs=1) as cp:
            # small const DMA leads the scalar ring
            cpk = cp.tile([128, 436], f16)
            nc.scalar.dma_start(cpk[:], cpkd[:])
            warm = cp.tile([128, 512], f16)
            w1t = cpk[ds(0, 120), ds(0, 64)]
            idt = cpk[:, ds(64, 128)]
            w2r = cpk[:, ds(192, 240)]
            b1t = cpk[:, ds(432, 1)]
            b2c = cpk[ds(0, 120), ds(433, 2)]

            # per-cycle x+aTlo blocks alternate across both HWDGE rings in
            # need order; aT mc1-half pairs trail so pa[0] completes early
            # and phase-3 mc0 hides under the mc1 stream.
            xb = [cp.tile([128, 6144], f8, name=f"xb_{s}") for s in range(NS)]
            ath = [cp.tile([128, 4096], f8, name=f"ath_{t}")
                   for t in range(NS // 2)]
            for s in range(NS):
                eng = nc.sync if s % 2 == 0 else nc.scalar
                if s < 2:
                    eng.dma_start(xb[s][:, ds(0, 4096)],
                                  xnb[ds(s * 128, 128), ds(0, 4096)])
                    eng.dma_start(xb[s][:, ds(4096, 2048)],
                                  xnb[ds(s * 128, 128), ds(4096, 2048)])
                else:
                    eng.dma_start(xb[s][:], xnb[ds(s * 128, 128), :])
            for t in range(NS // 2):
                eng = nc.sync if t % 2 == 0 else nc.scalar
                eng.dma_start(ath[t][:], athd[ds(128 * t, 128), :])

            z2 = cp.tile([120, 512], f16)
            hsb = cp.tile([128, N], f16)

            with tc.tile_pool(name="pa", bufs=1, space="PSUM") as ps2:
                pa = [ps2.tile([128, 512], f32, name=f"pa_{i}")
                      for i in range(2)]

                p1s = {}

                def emit_p1(s, ps1):
                    p1 = ps1.tile([128, 512], f32, name="p1")
                    p1s[s] = p1
                    for fh in range(2):
                        for b in range(NB):
                            nc.tensor.matmul(
                                p1[ds(32 * b, 32), :],
                                w1t[:, ds(32 * fh, 32)],
                                xb[s][ds(0, 120),
                                      ds(b * 1024 + fh * 512, 512)],
                                start=(fh == 0), stop=(fh == 1),
                                tile_position=(0, 32 * b))

                def emit_t(s, pst, hts):
                    hT = hts.tile([128, 512], f16, name="hT")
                    nc.vector.tensor_copy(hT[:], p1s[s][:])
                    pt = pst.tile([128, 512], f16, name="pt")
                    for m in range(4):
                        nc.tensor.transpose(
                            pt[:, ts(m, 128)], hT[:, ts(m, 128)], idt)
                    nc.vector.tensor_copy(hsb[:, ds(512 * s, 512)], pt[:])

                def emit_p2(s, mc):
                    for m in range(4):
                        kt = 4 * s + m
                        if mc == 0:
                            mov = xb[s][:, ds(4096 + 512 * m, 512)]
                        else:
                            mov = ath[s // 2][:, ds(2048 * (s % 2) +
                                                    512 * m, 512)]
                        nc.tensor.matmul(
                            pa[mc][:], hsb[:, ds(128 * kt, 128)], mov,
                            start=(kt == 0), stop=(kt == 31))

                with tc.tile_pool(name="ps1", bufs=2, space="PSUM") as ps1, \
                     tc.tile_pool(name="pst", bufs=2, space="PSUM") as pst, \
                     tc.tile_pool(name="hts", bufs=2) as hts:
                    # HAM warmup: keep PE busy from ~5us so the clock gate
                    # opens (K=8/8) before the first real matmul
                    nc.vector.memset(warm[:], 0.0)
                    nc.vector.memset(z2[:], 0.0)
                    for w in range(NWARM):
                        pw = ps1.tile([128, 512], f32, name="p1")
                        nc.tensor.matmul(pw[:], warm[:, 0:128], warm[:],
                                         start=True, stop=True)

                    emit_p1(0, ps1)
                    emit_t(0, pst, hts)
                    for s in range(1, NS):
                        emit_p1(s, ps1)
                        emit_p2(s - 1, 0)
                        emit_t(s, pst, hts)
                    emit_p2(0, 1)
                    emit_p2(NS - 1, 0)

                # sweep B (pa[1]) with phase 3 of mc0 interleaved under it
                with tc.tile_pool(name="rs", bufs=2) as rs, \
                     tc.tile_pool(name="os", bufs=2) as osb, \
                     tc.tile_pool(name="ps3", bufs=6, space="PSUM") as ps3:

                    def emit_relu(mc):
                        r = rs.tile([128, 512], f16, name="r")
                        nc.scalar.activation(r[:], pa[mc][:], relu,
                                             bias=b1t)
                        return r

                    def emit_p3(mc, lh, r):
                        o = osb.tile([120, 2048], f16, name="o")
                        p3s = []
                        for b in range(NB):
                            p3 = ps3.tile([120, 512], f32, name="p3")
                            nc.tensor.matmul(
                                p3[:],
                                w2r[ds(32 * b, 32), ds(120 * lh, 120)],
                                r[ds(32 * b, 32), :],
                                start=True, stop=True,
                                tile_position=(32 * b, 0))
                            p3s.append(p3)
                        for b in range(2):
                            nc.vector.scalar_tensor_tensor(
                                o[:, ds(512 * b, 512)], p3s[b][:],
                                b2c[:, ds(lh, 1)], z2[:],
                                op0=add, op1=byp)
                        nc.sync.dma_start(
                            outp[:, ds((mc * 2 + lh) * 2048, 1024)],
                            o[:, ds(0, 1024)])
                        for b in range(2, NB):
                            nc.scalar.activation(
                                o[:, ds(512 * b, 512)], p3s[b][:], ident,
                                bias=b2c[:, ds(lh, 1)])
                        nc.scalar.dma_start(
                            outp[:, ds((mc * 2 + lh) * 2048 + 1024, 1024)],
                            o[:, ds(1024, 1024)])

                    r0 = emit_relu(0)
                    emit_p2(1, 1)
                    emit_p3(0, 0, r0)
                    emit_p2(2, 1)
                    emit_p3(0, 1, r0)
                    for s in range(3, NS):
                        emit_p2(s, 1)
                    r1 = emit_relu(1)
                    emit_p3(1, 0, r1)
                    emit_p3(1, 1, r1)

    nc.compile()
    return nc


def kernel(x, a, W1, b1, W2, b2):
    global last_exec_time_ns, last_profile_json
    import ml_dtypes
    from concourse.bass_utils import run_bass_kernel_spmd

    if "nc" not in _cache:
        _cache["nc"] = _build()
    nc = _cache["nc"]

    x = np.asarray(x, np.float32)
    a = np.asarray(a, np.float32)
    W1 = np.asarray(W1, np.float32)
    b1 = np.asarray(b1, np.float32)
    W2 = np.asarray(W2, np.float32)
    b2 = np.asarray(b2, np.float32)

    # x part: [s, p<120, b*1024 + fh*512 + nl] = x[4g+b, 512s + nl, 120fh + p]
    xg = []
    for g in range(2):
        xpart = x[g * NB:(g + 1) * NB]                   # [4, 4096, 240]
        v = xpart.reshape(NB, NS, 512, 2, 120).transpose(1, 4, 0, 3, 2)
        xg.append(np.ascontiguousarray(v).reshape(NS, 120, 4096)
                  .astype(ml_dtypes.float8_e3m4))
    # a part (mc-major): aj[j][128k4 + p, 2048mc + 512q + c]
    #   = a[1024j + 512mc + c, 512k4 + 128q + p]
    aj = []
    for j in range(4):
        ajT = np.ascontiguousarray(a[j * NRC:(j + 1) * NRC, :].T)  # [4096,1024]
        v = ajT.reshape(8, 4, 128, 2, 512).transpose(0, 2, 3, 1, 4)
        aj.append(np.ascontiguousarray(v).reshape(1024, 4096)
                  .astype(ml_dtypes.float8_e3m4))
    # paired blocks per core variant
    xnbs = {}
    aths = {}
    for g in range(2):
        for j in range(4):
            blk = np.zeros((NS, 128, 6144), ml_dtypes.float8_e3m4)
            blk[:, 0:120, 0:4096] = xg[g]
            blk[:, :, 4096:6144] = aj[j].reshape(NS, 128, 4096)[:, :, 0:2048]
            xnbs[(g, j)] = np.ascontiguousarray(blk).reshape(NS * 128, 6144)
    for j in range(4):
        hi = aj[j].reshape(NS, 128, 4096)[:, :, 2048:4096]  # [8,128,2048]
        ath = hi.reshape(4, 2, 128, 2048).transpose(0, 2, 1, 3)
        aths[j] = np.ascontiguousarray(ath).reshape(512, 4096)
    cpkd = np.zeros((128, 436), np.float16)
    # w1: cpkd[p, fh*32 + h] = W1[120*fh + p, h]
    cpkd[0:120, 0:64] = W1.reshape(2, 120, 32).transpose(1, 0, 2) \
        .reshape(120, 64).astype(np.float16)
    cpkd[:, 64:192] = np.eye(128, dtype=np.float16)
    cpkd[:, 192:432] = np.tile(W2.astype(np.float16), (4, 1))
    cpkd[:, 432] = np.tile(b1, 4).astype(np.float16)
    cpkd[0:120, 433:435] = b2.reshape(2, 120).T.astype(np.float16)

    ins = []
    for c in range(8):
        g, j = c // 4, c % 4
        ins.append({"xnb": xnbs[(g, j)], "athd": aths[j], "cpkd": cpkd})

    trace = TRACE
    if trace:
        try:
            _install_ntff_hook()
        except Exception:
            trace = False
    r = run_bass_kernel_spmd(nc, ins, list(range(8)), trace=trace)
    last_exec_time_ns = r.exec_time_ns
    last_profile_json = r.profile_json

    res = np.empty((B, N, L), np.float32)
    for c in range(8):
        g, j = c // 4, c % 4
        # outp[p, mc, lh, b, c] -> out[4g+b, 1024j + 512mc + c, 120lh + p]
        arr = r.results[c]["outp"].reshape(120, 2, 2, NB, 512)
        res[g * NB:(g + 1) * NB, j * NRC:(j + 1) * NRC, :] = \
            arr.transpose(3, 1, 4, 2, 0).reshape(NB, NRC, L).astype(np.float32)
    return res


# revision 23
# speedup vs baseline: 1.0297x; 1.0297x over previous
"""GCN message-passing kernel for 8 trn2 NeuronCores.

Math (per reference): h = relu(a @ (x @ W1) + b1); out = h @ W2 + b2
Shapes: x [8,4096,240], a [4096,4096], W1 [240,32], W2 [32,240].

Sharding: 2x4 grid. Core c -> batch group g=c//4 (4 batches), output-row
group j=c%4 (1024 rows). W1/W2 fp16; x AND a host-converted to fp8-e3m4.
PSUM fp32; output fp16.

v3 design (baseline 71.9us -> v2 70.8 -> this):
- HAM warmup: 10 dummy matmuls on a zeroed tile during the NEFF preamble
  so the PE runs at 2.4GHz when real data lands.
- Col-tiled phase 1: 4 concurrent matmuls per (fh, nl) round via
  tile_position (one batch per 32-col strip).
- 8 sub-groups of 512 nodes; PE emission P1(s), P2A(s-1), T(s) keeps
  independent streams mixed (covers DVE handoffs and LDW latency).
- Phase 2 split into mc-major sweeps: sweep A accumulates pa[0] (rows
  0-511) inside the main loop; sweep B (pa[1]) runs after, interleaved
  with phase 3 of mc0 -- half of phase 3 hides under sweep B's matmuls.
  Same total LDW count (bass reloads weights per matmul anyway).
- Transposed phase 3: out[l-partitions, nodes] via 4 row-tiled concurrent
  matmuls (stationary W2 [32,120] per batch at tile_position (32b,0)), so
  b2 is a per-partition bias: one-op cast+bias on vector
  (scalar_tensor_tensor add/bypass) or scalar (Identity ACT with bias AP),
  split 2+2 per lh-block. relu+b1 via vector add/max against a zero tile
  (no Relu table -> no ACT table switches).
- DMA: sync ring = x stream (8 x 491KB) + outputs (4 x 491KB);
  scalar ring = W1/identity/b1 then aT stream (8 x 524KB);
  gpsimd (SWDGE) = small phase-3 consts only.
"""

import sys

if "/opt/trn_rl_repo" not in sys.path:
    sys.path.insert(0, "/opt/trn_rl_repo")

import numpy as np

B, N, F, H, L = 8, 4096, 240, 32, 240
NB = 4        # batches per core
NRC = 1024    # output rows per core
NS = 8        # node sub-groups per core (512 nodes each)
NWARM = 10
TRACE = False

_cache = {}
last_exec_time_ns = None
last_profile_json = None


def _install_ntff_hook():
    import types

    import antenv

    if "antenv.axon_hooks" in sys.modules:
        return
    mod = types.ModuleType("antenv.axon_hooks")
    _state = {"hook": None}
    mod.set_axon_ntff_profile_hook = lambda h: _state.__setitem__("hook", h)
    mod.get_axon_ntff_profile_hook = lambda: _state["hook"]
    sys.modules["antenv.axon_hooks"] = mod
    antenv.axon_hooks = mod
    from trn_agent_boot.trn_boot import _ntff_profile_via_ctypes

    mod.set_axon_ntff_profile_hook(
        _ntff_profile_via_ctypes("/opt/axon/libaxon_pjrt.so")
    )


def _build():
    import concourse.bass as bass
    import concourse.tile as tile
    from concourse import bacc, mybir

    f32 = mybir.dt.float32
    f16 = mybir.dt.float16
    f8 = mybir.dt.float8e3
    ts, ds = bass.ts, bass.ds
    add = mybir.AluOpType.add
    amax = mybir.AluOpType.max
    byp = mybir.AluOpType.bypass
    ident = mybir.ActivationFunctionType.Identity
    relu = mybir.ActivationFunctionType.Relu

    nc = bacc.Bacc("TRN2", target_bir_lowering=False, debug=False, num_devices=8)
    # xnb block s: cols [0:4096] x of sub-group s (rows 0:120, layout
    # b*1024 + fh*512 + nl), cols [4096:6144] / [6144:8192] the mc0 / mc1
    # halves of the same sub-group's aT block (512*q + c each) -- one
    # 1MB DMA per loop cycle.
    xnb = nc.dram_tensor("xnb", [NS * 128, 8192], f8, kind="ExternalInput").ap()
    # packed consts, all f16: cols [0:64] W1 (two fh halves), [64:192]
    # identity, [192:432] W2 tiled 4x vertically, [432] b1 (4x tiled),
    # [433:435] b2 halves
    cpkd = nc.dram_tensor("cpkd", [128, 436], f16, kind="ExternalInput").ap()
    # outp[p, ((mc*2+lh)*4 + b)*512 + c] = out[4g+b, 1024j + 512mc + c, 120lh + p]
    outp = nc.dram_tensor("outp", [120, 8192], f16, kind="ExternalOutput").ap()

    with tile.TileContext(nc) as tc:
        with tc.tile_pool(name="const", bufs=1) as cp:
            # small const DMA leads the scalar ring
            cpk = cp.tile([128, 436], f16)
            nc.scalar.dma_start(cpk[:], cpkd[:])
            warm = cp.tile([128, 512], f16)
            w1t = cpk[ds(0, 120), ds(0, 64)]
            idt = cpk[:, ds(64, 128)]
            w2r = cpk[:, ds(192, 240)]
            b1t = cpk[:, ds(432, 1)]
            b2c = cpk[ds(0, 120), ds(433, 2)]

            # per-cycle x+aT blocks alternate across both HWDGE rings in
            # need order (the first two split so phase 1 starts sooner)
            xb = [cp.tile([128, 8192], f8, name=f"xb_{s}") for s in range(NS)]
            for s in range(NS):
                eng = nc.sync if s % 2 == 0 else nc.scalar
                if s < 2:
                    eng.dma_start(xb[s][:, ds(0, 4096)],
                                  xnb[ds(s * 128, 128), ds(0, 4096)])
                    eng.dma_start(xb[s][:, ds(4096, 4096)],
                                  xnb[ds(s * 128, 128), ds(4096, 4096)])
                else:
                    eng.dma_start(xb[s][:], xnb[ds(s * 128, 128), :])

            zf = cp.tile([128, 512], f32)
            z2 = cp.tile([120, 512], f16)
            hsb = cp.tile([128, N], f16)

            with tc.tile_pool(name="pa", bufs=1, space="PSUM") as ps2:
                pa = [ps2.tile([128, 512], f32, name=f"pa_{i}")
                      for i in range(2)]

                p1s = {}

                def emit_p1(s, ps1):
                    p1 = ps1.tile([128, 512], f32, name="p1")
                    p1s[s] = p1
                    for fh in range(2):
                        for b in range(NB):
                            nc.tensor.matmul(
                                p1[ds(32 * b, 32), :],
                                w1t[:, ds(32 * fh, 32)],
                                xb[s][ds(0, 120),
                                      ds(b * 1024 + fh * 512, 512)],
                                start=(fh == 0), stop=(fh == 1),
                                tile_position=(0, 32 * b))

                def emit_t(s, pst, hts):
                    hT = hts.tile([128, 512], f16, name="hT")
                    nc.vector.tensor_copy(hT[:], p1s[s][:])
                    pt = pst.tile([128, 512], f16, name="pt")
                    for m in range(4):
                        nc.tensor.transpose(
                            pt[:, ts(m, 128)], hT[:, ts(m, 128)], idt)
                    nc.vector.tensor_copy(hsb[:, ds(512 * s, 512)], pt[:])

                def emit_p2(s, mc):
                    for m in range(4):
                        kt = 4 * s + m
                        mov = xb[s][:, ds(4096 + 2048 * mc + 512 * m,
                                          512)]
                        nc.tensor.matmul(
                            pa[mc][:], hsb[:, ds(128 * kt, 128)], mov,
                            start=(kt == 0), stop=(kt == 31))

                with tc.tile_pool(name="ps1", bufs=2, space="PSUM") as ps1, \
                     tc.tile_pool(name="pst", bufs=2, space="PSUM") as pst, \
                     tc.tile_pool(name="hts", bufs=2) as hts:
                    # HAM warmup: keep PE busy from ~5us so the clock gate
                    # opens (K=8/8) before the first real matmul
                    nc.vector.memset(warm[:], 0.0)
                    nc.vector.memset(z2[:], 0.0)
                    nc.vector.memset(zf[:], 0.0)
                    for w in range(NWARM):
                        pw = ps1.tile([128, 512], f32, name="p1")
                        nc.tensor.matmul(pw[:], warm[:, 0:128], warm[:],
                                         start=True, stop=True)

                    emit_p1(0, ps1)
                    emit_t(0, pst, hts)
                    for s in range(1, NS):
                        emit_p1(s, ps1)
                        emit_p2(s - 1, 0)
                        emit_p2(s - 1, 1)
                        emit_t(s, pst, hts)
                    emit_p2(NS - 1, 0)
                    emit_p2(NS - 1, 1)

                # sweep B (pa[1]) with phase 3 of mc0 interleaved under it
                with tc.tile_pool(name="rs", bufs=2) as rs, \
                     tc.tile_pool(name="os", bufs=2) as osb, \
                     tc.tile_pool(name="ps3", bufs=6, space="PSUM") as ps3:

                    def emit_relu(mc):
                        r = rs.tile([128, 512], f16, name="r")
                        if mc == 0:
                            nc.scalar.activation(r[:], pa[mc][:], relu,
                                                 bias=b1t)
                        else:
                            nc.vector.scalar_tensor_tensor(
                                r[:], pa[mc][:], b1t, zf[:],
                                op0=add, op1=amax)
                        return r

                    def emit_p3(mc, lh, r):
                        o = osb.tile([120, 2048], f16, name="o")
                        p3s = []
                        for b in range(NB):
                            p3 = ps3.tile([120, 512], f32, name="p3")
                            nc.tensor.matmul(
                                p3[:],
                                w2r[ds(32 * b, 32), ds(120 * lh, 120)],
                                r[ds(32 * b, 32), :],
                                start=True, stop=True,
                                tile_position=(32 * b, 0))
                            p3s.append(p3)
                        for b in range(2):
                            nc.vector.scalar_tensor_tensor(
                                o[:, ds(512 * b, 512)], p3s[b][:],
                                b2c[:, ds(lh, 1)], z2[:],
                                op0=add, op1=byp)
                        nc.sync.dma_start(
                            outp[:, ds((mc * 2 + lh) * 2048, 1024)],
                            o[:, ds(0, 1024)])
                        for b in range(2, NB):
                            nc.scalar.activation(
                                o[:, ds(512 * b, 512)], p3s[b][:], ident,
                                bias=b2c[:, ds(lh, 1)])
                        nc.scalar.dma_start(
                            outp[:, ds((mc * 2 + lh) * 2048 + 1024, 1024)],
                            o[:, ds(1024, 1024)])

                    r0 = emit_relu(0)
                    r1 = emit_relu(1)
                    emit_p3(0, 0, r0)
                    emit_p3(1, 0, r1)
                    emit_p3(0, 1, r0)
                    emit_p3(1, 1, r1)

    nc.compile()
    return nc


def kernel(x, a, W1, b1, W2, b2):
    global last_exec_time_ns, last_profile_json
    import ml_dtypes
    from concourse.bass_utils import run_bass_kernel_spmd

    if "nc" not in _cache:
        _cache["nc"] = _build()
    nc = _cache["nc"]

    x = np.asarray(x, np.float32)
    a = np.asarray(a, np.float32)
    W1 = np.asarray(W1, np.float32)
    b1 = np.asarray(b1, np.float32)
    W2 = np.asarray(W2, np.float32)
    b2 = np.asarray(b2, np.float32)

    # x part: [s, p<120, b*1024 + fh*512 + nl] = x[4g+b, 512s + nl, 120fh + p]
    xg = []
    for g in range(2):
        xpart = x[g * NB:(g + 1) * NB]                   # [4, 4096, 240]
        v = xpart.reshape(NB, NS, 512, 2, 120).transpose(1, 4, 0, 3, 2)
        xg.append(np.ascontiguousarray(v).reshape(NS, 120, 4096)
                  .astype(ml_dtypes.float8_e3m4))
    # a part (mc-major): aj[j][128k4 + p, 2048mc + 512q + c]
    #   = a[1024j + 512mc + c, 512k4 + 128q + p]
    aj = []
    for j in range(4):
        ajT = np.ascontiguousarray(a[j * NRC:(j + 1) * NRC, :].T)  # [4096,1024]
        v = ajT.reshape(8, 4, 128, 2, 512).transpose(0, 2, 3, 1, 4)
        aj.append(np.ascontiguousarray(v).reshape(1024, 4096)
                  .astype(ml_dtypes.float8_e3m4))
    # paired blocks per core variant
    xnbs = {}
    for g in range(2):
        for j in range(4):
            blk = np.zeros((NS, 128, 8192), ml_dtypes.float8_e3m4)
            blk[:, 0:120, 0:4096] = xg[g]
            blk[:, :, 4096:8192] = aj[j].reshape(NS, 128, 4096)
            xnbs[(g, j)] = np.ascontiguousarray(blk).reshape(NS * 128, 8192)
    cpkd = np.zeros((128, 436), np.float16)
    # w1: cpkd[p, fh*32 + h] = W1[120*fh + p, h]
    cpkd[0:120, 0:64] = W1.reshape(2, 120, 32).transpose(1, 0, 2) \
        .reshape(120, 64).astype(np.float16)
    cpkd[:, 64:192] = np.eye(128, dtype=np.float16)
    cpkd[:, 192:432] = np.tile(W2.astype(np.float16), (4, 1))
    cpkd[:, 432] = np.tile(b1, 4).astype(np.float16)
    cpkd[0:120, 433:435] = b2.reshape(2, 120).T.astype(np.float16)

    ins = []
    for c in range(8):
        g, j = c // 4, c % 4
        ins.append({"xnb": xnbs[(g, j)], "cpkd": cpkd})

    trace = TRACE
    if trace:
        try:
            _install_ntff_hook()
        except Exception:
            trace = False
    r = run_bass_kernel_spmd(nc, ins, list(range(8)), trace=trace)
    last_exec_time_ns = r.exec_time_ns
    last_profile_json = r.profile_json

    res = np.empty((B, N, L), np.float32)
    for c in range(8):
        g, j = c // 4, c % 4
        # outp[p, mc, lh, b, c] -> out[4g+b, 1024j + 512mc + c, 120lh + p]
        arr = r.results[c]["outp"].reshape(120, 2, 2, NB, 512)
        res[g * NB:(g + 1) * NB, j * NRC:(j + 1) * NRC, :] = \
            arr.transpose(3, 1, 4, 2, 0).reshape(NB, NRC, L).astype(np.float32)
    return res


# revision 24
# speedup vs baseline: 1.1136x; 1.0815x over previous
"""GCN message-passing kernel for 8 trn2 NeuronCores.

Math (per reference): h = relu(a @ (x @ W1) + b1); out = h @ W2 + b2
Shapes: x [8,4096,240], a [4096,4096], W1 [240,32], W2 [32,240].

Sharding: 2x4 grid. Core c -> batch group g=c//4 (4 batches), output-row
group j=c%4 (1024 rows). W1/W2 fp16; x AND a host-converted to fp8-e3m4.
PSUM fp32; output fp16.

v3 design (baseline 71.9us -> v2 70.8 -> this):
- HAM warmup: 10 dummy matmuls on a zeroed tile during the NEFF preamble
  so the PE runs at 2.4GHz when real data lands.
- Col-tiled phase 1: 4 concurrent matmuls per (fh, nl) round via
  tile_position (one batch per 32-col strip).
- 8 sub-groups of 512 nodes; PE emission P1(s), P2A(s-1), T(s) keeps
  independent streams mixed (covers DVE handoffs and LDW latency).
- Phase 2 split into mc-major sweeps: sweep A accumulates pa[0] (rows
  0-511) inside the main loop; sweep B (pa[1]) runs after, interleaved
  with phase 3 of mc0 -- half of phase 3 hides under sweep B's matmuls.
  Same total LDW count (bass reloads weights per matmul anyway).
- Transposed phase 3: out[l-partitions, nodes] via 4 row-tiled concurrent
  matmuls (stationary W2 [32,120] per batch at tile_position (32b,0)), so
  b2 is a per-partition bias: one-op cast+bias on vector
  (scalar_tensor_tensor add/bypass) or scalar (Identity ACT with bias AP),
  split 2+2 per lh-block. relu+b1 via vector add/max against a zero tile
  (no Relu table -> no ACT table switches).
- DMA: sync ring = x stream (8 x 491KB) + outputs (4 x 491KB);
  scalar ring = W1/identity/b1 then aT stream (8 x 524KB);
  gpsimd (SWDGE) = small phase-3 consts only.
"""

import sys

if "/opt/trn_rl_repo" not in sys.path:
    sys.path.insert(0, "/opt/trn_rl_repo")

import numpy as np

B, N, F, H, L = 8, 4096, 240, 32, 240
NB = 4        # batches per core
NRC = 1024    # output rows per core
NS = 8        # node sub-groups per core (512 nodes each)
NWARM = 10
TRACE = False

_cache = {}
last_exec_time_ns = None
last_profile_json = None


def _install_ntff_hook():
    import types

    import antenv

    if "antenv.axon_hooks" in sys.modules:
        return
    mod = types.ModuleType("antenv.axon_hooks")
    _state = {"hook": None}
    mod.set_axon_ntff_profile_hook = lambda h: _state.__setitem__("hook", h)
    mod.get_axon_ntff_profile_hook = lambda: _state["hook"]
    sys.modules["antenv.axon_hooks"] = mod
    antenv.axon_hooks = mod
    from trn_agent_boot.trn_boot import _ntff_profile_via_ctypes

    mod.set_axon_ntff_profile_hook(
        _ntff_profile_via_ctypes("/opt/axon/libaxon_pjrt.so")
    )


def _build():
    import concourse.bass as bass
    import concourse.tile as tile
    from concourse import bacc, mybir

    f32 = mybir.dt.float32
    f16 = mybir.dt.float16
    f8 = mybir.dt.float8e3
    ts, ds = bass.ts, bass.ds
    add = mybir.AluOpType.add
    amax = mybir.AluOpType.max
    byp = mybir.AluOpType.bypass
    ident = mybir.ActivationFunctionType.Identity
    relu = mybir.ActivationFunctionType.Relu

    nc = bacc.Bacc("TRN2", target_bir_lowering=False, debug=False, num_devices=8)
    # xnb block s: cols [0:4096] x of sub-group s (rows 0:120, layout
    # b*1024 + fh*512 + nl), cols [4096:6144] the mc0 half of the same
    # sub-group's aT block (512*q + c) -- one DMA per loop cycle.
    xnb = nc.dram_tensor("xnb", [NS * 128, 6144], f8, kind="ExternalInput").ap()
    # athd pair t: cols [0:2048]/[2048:4096] = mc1 halves of sub-groups
    # 2t / 2t+1 (512*q + c each)
    athd = nc.dram_tensor("athd", [512, 4096], f8, kind="ExternalInput").ap()
    # packed consts, all f16: cols [0:64] W1 (two fh halves), [64:192]
    # identity, [192:432] W2 tiled 4x vertically, [432] b1 (4x tiled),
    # [433:435] b2 halves
    cpkd = nc.dram_tensor("cpkd", [128, 436], f16, kind="ExternalInput").ap()
    # outp[p, ((mc*2+lh)*4 + b)*512 + c] = out[4g+b, 1024j + 512mc + c, 120lh + p]
    outp = nc.dram_tensor("outp", [120, 8192], f16, kind="ExternalOutput").ap()

    with tile.TileContext(nc) as tc:
        with tc.tile_pool(name="const", bufs=1) as cp:
            # small const DMA leads the scalar ring
            cpk = cp.tile([128, 436], f16)
            nc.scalar.dma_start(cpk[:], cpkd[:])
            warm = cp.tile([128, 512], f16)
            w1t = cpk[ds(0, 120), ds(0, 64)]
            idt = cpk[:, ds(64, 128)]
            w2r = cpk[:, ds(192, 240)]
            b1t = cpk[:, ds(432, 1)]
            b2c = cpk[ds(0, 120), ds(433, 2)]

            # per-cycle x+aTlo blocks alternate across both HWDGE rings
            # in need order; aT mc1-half pairs trail so pa[0] completes
            # early and phase-3 mc0 hides under the mc1 stream.
            xb = [cp.tile([128, 6144], f8, name=f"xb_{s}") for s in range(NS)]
            ath = [cp.tile([128, 4096], f8, name=f"ath_{t}")
                   for t in range(NS // 2)]
            for s in range(NS):
                eng = nc.sync if s % 2 == 0 else nc.scalar
                if s < 2:
                    eng.dma_start(xb[s][:, ds(0, 4096)],
                                  xnb[ds(s * 128, 128), ds(0, 4096)])
                    eng.dma_start(xb[s][:, ds(4096, 2048)],
                                  xnb[ds(s * 128, 128), ds(4096, 2048)])
                else:
                    eng.dma_start(xb[s][:], xnb[ds(s * 128, 128), :])
            for t in range(NS // 2):
                eng = nc.sync if t % 2 == 0 else nc.scalar
                eng.dma_start(ath[t][:], athd[ds(128 * t, 128), :])

            zf = cp.tile([128, 512], f32)
            z2 = cp.tile([120, 512], f16)
            hsb = cp.tile([128, N], f16)

            with tc.tile_pool(name="pa", bufs=1, space="PSUM") as ps2:
                pa = [ps2.tile([128, 512], f32, name=f"pa_{i}")
                      for i in range(2)]

                p1s = {}

                def emit_p1(s, ps1):
                    p1 = ps1.tile([128, 512], f32, name="p1")
                    p1s[s] = p1
                    for fh in range(2):
                        for b in range(NB):
                            nc.tensor.matmul(
                                p1[ds(32 * b, 32), :],
                                w1t[:, ds(32 * fh, 32)],
                                xb[s][ds(0, 120),
                                      ds(b * 1024 + fh * 512, 512)],
                                start=(fh == 0), stop=(fh == 1),
                                tile_position=(0, 32 * b))

                def emit_t(s, pst, hts):
                    hT = hts.tile([128, 512], f16, name="hT")
                    nc.vector.tensor_copy(hT[:], p1s[s][:])
                    pt = pst.tile([128, 512], f16, name="pt")
                    for m in range(4):
                        nc.tensor.transpose(
                            pt[:, ts(m, 128)], hT[:, ts(m, 128)], idt)
                    nc.vector.tensor_copy(hsb[:, ds(512 * s, 512)], pt[:])

                def emit_p2(s, mc):
                    for m in range(4):
                        kt = 4 * s + m
                        if mc == 0:
                            mov = xb[s][:, ds(4096 + 512 * m, 512)]
                        else:
                            mov = ath[s // 2][:, ds(2048 * (s % 2) +
                                                    512 * m, 512)]
                        nc.tensor.matmul(
                            pa[mc][:], hsb[:, ds(128 * kt, 128)], mov,
                            start=(kt == 0), stop=(kt == 31))

                with tc.tile_pool(name="ps1", bufs=2, space="PSUM") as ps1, \
                     tc.tile_pool(name="pst", bufs=2, space="PSUM") as pst, \
                     tc.tile_pool(name="hts", bufs=2) as hts:
                    # HAM warmup: keep PE busy from ~5us so the clock gate
                    # opens (K=8/8) before the first real matmul
                    nc.vector.memset(warm[:], 0.0)
                    nc.vector.memset(z2[:], 0.0)
                    nc.vector.memset(zf[:], 0.0)
                    for w in range(NWARM):
                        pw = ps1.tile([128, 512], f32, name="p1")
                        nc.tensor.matmul(pw[:], warm[:, 0:128], warm[:],
                                         start=True, stop=True)

                    emit_p1(0, ps1)
                    emit_t(0, pst, hts)
                    for s in range(1, NS):
                        emit_p1(s, ps1)
                        emit_p2(s - 1, 0)
                        emit_t(s, pst, hts)
                    emit_p2(0, 1)
                    emit_p2(NS - 1, 0)

                # sweep B (pa[1]) with phase 3 of mc0 interleaved under it
                with tc.tile_pool(name="rs", bufs=2) as rs, \
                     tc.tile_pool(name="os", bufs=2) as osb, \
                     tc.tile_pool(name="ps3", bufs=6, space="PSUM") as ps3:

                    def emit_relu(mc):
                        r = rs.tile([128, 512], f16, name="r")
                        nc.scalar.activation(r[:], pa[mc][:], relu,
                                             bias=b1t)
                        return r

                    def emit_p3(mc, lh, r):
                        o = osb.tile([120, 2048], f16, name="o")
                        p3s = []
                        for b in range(NB):
                            p3 = ps3.tile([120, 512], f32, name="p3")
                            nc.tensor.matmul(
                                p3[:],
                                w2r[ds(32 * b, 32), ds(120 * lh, 120)],
                                r[ds(32 * b, 32), :],
                                start=True, stop=True,
                                tile_position=(32 * b, 0))
                            p3s.append(p3)
                        for b in range(2):
                            nc.vector.scalar_tensor_tensor(
                                o[:, ds(512 * b, 512)], p3s[b][:],
                                b2c[:, ds(lh, 1)], z2[:],
                                op0=add, op1=byp)
                        nc.sync.dma_start(
                            outp[:, ds((mc * 2 + lh) * 2048, 1024)],
                            o[:, ds(0, 1024)])
                        for b in range(2, NB):
                            nc.scalar.activation(
                                o[:, ds(512 * b, 512)], p3s[b][:], ident,
                                bias=b2c[:, ds(lh, 1)])
                        nc.scalar.dma_start(
                            outp[:, ds((mc * 2 + lh) * 2048 + 1024, 1024)],
                            o[:, ds(1024, 1024)])

                    r0 = emit_relu(0)
                    emit_p2(1, 1)
                    emit_p3(0, 0, r0)
                    emit_p2(2, 1)
                    emit_p3(0, 1, r0)
                    for s in range(3, NS):
                        emit_p2(s, 1)
                    r1 = emit_relu(1)
                    emit_p3(1, 0, r1)
                    emit_p3(1, 1, r1)

    nc.compile()
    return nc


def kernel(x, a, W1, b1, W2, b2):
    global last_exec_time_ns, last_profile_json
    import ml_dtypes
    from concourse.bass_utils import run_bass_kernel_spmd

    if "nc" not in _cache:
        _cache["nc"] = _build()
    nc = _cache["nc"]

    x = np.asarray(x, np.float32)
    a = np.asarray(a, np.float32)
    W1 = np.asarray(W1, np.float32)
    b1 = np.asarray(b1, np.float32)
    W2 = np.asarray(W2, np.float32)
    b2 = np.asarray(b2, np.float32)

    # x part: [s, p<120, b*1024 + fh*512 + nl] = x[4g+b, 512s + nl, 120fh + p]
    xg = []
    for g in range(2):
        xpart = x[g * NB:(g + 1) * NB]                   # [4, 4096, 240]
        v = xpart.reshape(NB, NS, 512, 2, 120).transpose(1, 4, 0, 3, 2)
        xg.append(np.ascontiguousarray(v).reshape(NS, 120, 4096)
                  .astype(ml_dtypes.float8_e3m4))
    # a part (mc-major): aj[j][128k4 + p, 2048mc + 512q + c]
    #   = a[1024j + 512mc + c, 512k4 + 128q + p]
    aj = []
    for j in range(4):
        ajT = np.ascontiguousarray(a[j * NRC:(j + 1) * NRC, :].T)  # [4096,1024]
        v = ajT.reshape(8, 4, 128, 2, 512).transpose(0, 2, 3, 1, 4)
        aj.append(np.ascontiguousarray(v).reshape(1024, 4096)
                  .astype(ml_dtypes.float8_e3m4))
    # paired blocks per core variant
    xnbs = {}
    aths = {}
    for g in range(2):
        for j in range(4):
            blk = np.zeros((NS, 128, 6144), ml_dtypes.float8_e3m4)
            blk[:, 0:120, 0:4096] = xg[g]
            blk[:, :, 4096:6144] = aj[j].reshape(NS, 128, 4096)[:, :, 0:2048]
            xnbs[(g, j)] = np.ascontiguousarray(blk).reshape(NS * 128, 6144)
    for j in range(4):
        hi = aj[j].reshape(NS, 128, 4096)[:, :, 2048:4096]  # [8,128,2048]
        ath = hi.reshape(4, 2, 128, 2048).transpose(0, 2, 1, 3)
        aths[j] = np.ascontiguousarray(ath).reshape(512, 4096)
    cpkd = np.zeros((128, 436), np.float16)
    # w1: cpkd[p, fh*32 + h] = W1[120*fh + p, h]
    cpkd[0:120, 0:64] = W1.reshape(2, 120, 32).transpose(1, 0, 2) \
        .reshape(120, 64).astype(np.float16)
    cpkd[:, 64:192] = np.eye(128, dtype=np.float16)
    cpkd[:, 192:432] = np.tile(W2.astype(np.float16), (4, 1))
    cpkd[:, 432] = np.tile(b1, 4).astype(np.float16)
    cpkd[0:120, 433:435] = b2.reshape(2, 120).T.astype(np.float16)

    ins = []
    for c in range(8):
        g, j = c // 4, c % 4
        ins.append({"xnb": xnbs[(g, j)], "athd": aths[j], "cpkd": cpkd})

    trace = TRACE
    if trace:
        try:
            _install_ntff_hook()
        except Exception:
            trace = False
    r = run_bass_kernel_spmd(nc, ins, list(range(8)), trace=trace)
    last_exec_time_ns = r.exec_time_ns
    last_profile_json = r.profile_json

    res = np.empty((B, N, L), np.float32)
    for c in range(8):
        g, j = c // 4, c % 4
        # outp[p, mc, lh, b, c] -> out[4g+b, 1024j + 512mc + c, 120lh + p]
        arr = r.results[c]["outp"].reshape(120, 2, 2, NB, 512)
        res[g * NB:(g + 1) * NB, j * NRC:(j + 1) * NRC, :] = \
            arr.transpose(3, 1, 4, 2, 0).reshape(NB, NRC, L).astype(np.float32)
    return res


# revision 31
# speedup vs baseline: 1.1169x; 1.0029x over previous
"""GCN message-passing kernel for 8 trn2 NeuronCores.

Math (per reference): h = relu(a @ (x @ W1) + b1); out = h @ W2 + b2
Shapes: x [8,4096,240], a [4096,4096], W1 [240,32], W2 [32,240].

Sharding: 2x4 grid. Core c -> batch group g=c//4 (4 batches), output-row
group j=c%4 (1024 rows). W1/W2 fp16; x AND a host-converted to fp8-e3m4.
PSUM fp32; output fp16.

v3 design (baseline 71.9us -> v2 70.8 -> this):
- HAM warmup: 10 dummy matmuls on a zeroed tile during the NEFF preamble
  so the PE runs at 2.4GHz when real data lands.
- Col-tiled phase 1: 4 concurrent matmuls per (fh, nl) round via
  tile_position (one batch per 32-col strip).
- 8 sub-groups of 512 nodes; PE emission P1(s), P2A(s-1), T(s) keeps
  independent streams mixed (covers DVE handoffs and LDW latency).
- Phase 2 split into mc-major sweeps: sweep A accumulates pa[0] (rows
  0-511) inside the main loop; sweep B (pa[1]) runs after, interleaved
  with phase 3 of mc0 -- half of phase 3 hides under sweep B's matmuls.
  Same total LDW count (bass reloads weights per matmul anyway).
- Transposed phase 3: out[l-partitions, nodes] via 4 row-tiled concurrent
  matmuls (stationary W2 [32,120] per batch at tile_position (32b,0)), so
  b2 is a per-partition bias: one-op cast+bias on vector
  (scalar_tensor_tensor add/bypass) or scalar (Identity ACT with bias AP),
  split 2+2 per lh-block. relu+b1 via vector add/max against a zero tile
  (no Relu table -> no ACT table switches).
- DMA: sync ring = x stream (8 x 491KB) + outputs (4 x 491KB);
  scalar ring = W1/identity/b1 then aT stream (8 x 524KB);
  gpsimd (SWDGE) = small phase-3 consts only.
"""

import sys

if "/opt/trn_rl_repo" not in sys.path:
    sys.path.insert(0, "/opt/trn_rl_repo")

import numpy as np

B, N, F, H, L = 8, 4096, 240, 32, 240
NB = 4        # batches per core
NRC = 1024    # output rows per core
NS = 8        # node sub-groups per core (512 nodes each)
NWARM = 10
TRACE = False

_cache = {}
last_exec_time_ns = None
last_profile_json = None


def _install_ntff_hook():
    import types

    import antenv

    if "antenv.axon_hooks" in sys.modules:
        return
    mod = types.ModuleType("antenv.axon_hooks")
    _state = {"hook": None}
    mod.set_axon_ntff_profile_hook = lambda h: _state.__setitem__("hook", h)
    mod.get_axon_ntff_profile_hook = lambda: _state["hook"]
    sys.modules["antenv.axon_hooks"] = mod
    antenv.axon_hooks = mod
    from trn_agent_boot.trn_boot import _ntff_profile_via_ctypes

    mod.set_axon_ntff_profile_hook(
        _ntff_profile_via_ctypes("/opt/axon/libaxon_pjrt.so")
    )


def _build():
    import concourse.bass as bass
    import concourse.tile as tile
    from concourse import bacc, mybir

    f32 = mybir.dt.float32
    f16 = mybir.dt.float16
    f8 = mybir.dt.float8e3
    ts, ds = bass.ts, bass.ds
    add = mybir.AluOpType.add
    byp = mybir.AluOpType.bypass
    ident = mybir.ActivationFunctionType.Identity
    relu = mybir.ActivationFunctionType.Relu

    nc = bacc.Bacc("TRN2", target_bir_lowering=False, debug=False, num_devices=8)
    # xnb block s: cols [0:4096] x of sub-group s (rows 0:120, layout
    # b*1024 + fh*512 + nl), cols [4096:6144] the mc0 half of the same
    # sub-group's aT block (512*q + c) -- one DMA per loop cycle.
    xnb = nc.dram_tensor("xnb", [NS * 128, 6144], f8, kind="ExternalInput").ap()
    # athd pair t: cols [0:2048]/[2048:4096] = mc1 halves of sub-groups
    # 2t / 2t+1 (512*q + c each)
    athd = nc.dram_tensor("athd", [512, 4096], f8, kind="ExternalInput").ap()
    # packed consts, all f16: cols [0:64] W1 (two fh halves), [64:192]
    # identity, [192:432] W2 tiled 4x vertically, [432] b1 (4x tiled),
    # [433:435] b2 halves
    cpkd = nc.dram_tensor("cpkd", [128, 436], f16, kind="ExternalInput").ap()
    # outp[p, ((mc*2+lh)*4 + b)*512 + c] = out[4g+b, 1024j + 512mc + c, 120lh + p]
    outp = nc.dram_tensor("outp", [120, 8192], f16, kind="ExternalOutput").ap()

    with tile.TileContext(nc) as tc:
        with tc.tile_pool(name="const", bufs=1) as cp:
            # small const DMA leads the scalar ring
            cpk = cp.tile([128, 436], f16)
            nc.scalar.dma_start(cpk[:], cpkd[:])
            warm = cp.tile([128, 512], f16)
            w1t = cpk[ds(0, 120), ds(0, 64)]
            idt = cpk[:, ds(64, 128)]
            w2r = cpk[:, ds(192, 240)]
            b1t = cpk[:, ds(432, 1)]
            b2c = cpk[ds(0, 120), ds(433, 2)]

            # per-cycle x+aTlo blocks alternate across both HWDGE rings
            # in need order; aT mc1-half pairs trail so pa[0] completes
            # early and phase-3 mc0 hides under the mc1 stream.
            xb = [cp.tile([128, 6144], f8, name=f"xb_{s}") for s in range(NS)]
            ath = [cp.tile([128, 4096], f8, name=f"ath_{t}")
                   for t in range(NS // 2)]
            for s in range(NS):
                eng = nc.sync if s % 2 == 0 else nc.scalar
                if s < 2:
                    eng.dma_start(xb[s][:, ds(0, 2048)],
                                  xnb[ds(s * 128, 128), ds(0, 2048)])
                    eng.dma_start(xb[s][:, ds(2048, 2048)],
                                  xnb[ds(s * 128, 128), ds(2048, 2048)])
                    eng.dma_start(xb[s][:, ds(4096, 2048)],
                                  xnb[ds(s * 128, 128), ds(4096, 2048)])
                else:
                    eng.dma_start(xb[s][:], xnb[ds(s * 128, 128), :])
            for t in range(NS // 2):
                eng = nc.sync if t % 2 == 0 else nc.scalar
                eng.dma_start(ath[t][:], athd[ds(128 * t, 128), :])

            z2 = cp.tile([120, 1024], f16)
            hsb = cp.tile([128, N], f16)

            with tc.tile_pool(name="pa", bufs=1, space="PSUM") as ps2:
                pa = [ps2.tile([128, 512], f32, name=f"pa_{i}")
                      for i in range(2)]

                p1s = {}

                def emit_p1(s, ps1):
                    p1 = ps1.tile([128, 512], f32, name="p1")
                    p1s[s] = p1
                    for fh in range(2):
                        for b in range(NB):
                            nc.tensor.matmul(
                                p1[ds(32 * b, 32), :],
                                w1t[:, ds(32 * fh, 32)],
                                xb[s][ds(0, 120),
                                      ds(fh * 2048 + b * 512, 512)],
                                start=(fh == 0), stop=(fh == 1),
                                tile_position=(0, 32 * b))

                def emit_t(s, pst, hts):
                    hT = hts.tile([128, 512], f16, name="hT")
                    nc.vector.tensor_copy(hT[:], p1s[s][:])
                    pt = pst.tile([128, 512], f16, name="pt")
                    for m in range(4):
                        nc.tensor.transpose(
                            pt[:, ts(m, 128)], hT[:, ts(m, 128)], idt)
                    nc.vector.tensor_copy(hsb[:, ds(512 * s, 512)], pt[:])

                def emit_p2(s, mc):
                    for m in range(4):
                        kt = 4 * s + m
                        if mc == 0:
                            mov = xb[s][:, ds(4096 + 512 * m, 512)]
                        else:
                            mov = ath[s // 2][:, ds(2048 * (s % 2) +
                                                    512 * m, 512)]
                        nc.tensor.matmul(
                            pa[mc][:], hsb[:, ds(128 * kt, 128)], mov,
                            start=(kt == 0), stop=(kt == 31))

                with tc.tile_pool(name="ps1", bufs=2, space="PSUM") as ps1, \
                     tc.tile_pool(name="pst", bufs=2, space="PSUM") as pst, \
                     tc.tile_pool(name="hts", bufs=2) as hts:
                    # HAM warmup: keep PE busy from ~5us so the clock gate
                    # opens (K=8/8) before the first real matmul
                    nc.vector.memset(warm[:], 0.0)
                    nc.vector.memset(z2[:], 0.0)
                    for w in range(NWARM):
                        pw = ps1.tile([128, 512], f32, name="p1")
                        nc.tensor.matmul(pw[:], warm[:, 0:128], warm[:],
                                         start=True, stop=True)

                    emit_p1(0, ps1)
                    emit_t(0, pst, hts)
                    for s in range(1, NS):
                        emit_p1(s, ps1)
                        emit_p2(s - 1, 0)
                        emit_t(s, pst, hts)
                    emit_p2(0, 1)
                    emit_p2(NS - 1, 0)

                # sweep B (pa[1]) with phase 3 of mc0 interleaved under it
                with tc.tile_pool(name="rs", bufs=2) as rs, \
                     tc.tile_pool(name="os", bufs=2) as osb, \
                     tc.tile_pool(name="ps3", bufs=3, space="PSUM") as ps3:

                    def emit_relu(mc):
                        r = rs.tile([128, 512], f16, name="r")
                        nc.scalar.activation(r[:], pa[mc][:], relu,
                                             bias=b1t)
                        return r

                    def emit_p3(mc, lh, r):
                        o = osb.tile([120, 2048], f16, name="o")
                        p3s = []
                        for half in range(2):
                            p3 = ps3.tile([120, 1024], f32, name="p3")
                            for u in range(2):
                                b = 2 * half + u
                                nc.tensor.matmul(
                                    p3[:, ds(512 * u, 512)],
                                    w2r[ds(32 * b, 32), ds(120 * lh, 120)],
                                    r[ds(32 * b, 32), :],
                                    start=True, stop=True,
                                    tile_position=(32 * b, 0))
                            p3s.append(p3)
                        nc.vector.scalar_tensor_tensor(
                            o[:, ds(0, 1024)], p3s[0][:],
                            b2c[:, ds(lh, 1)], z2[:],
                            op0=add, op1=byp)
                        nc.sync.dma_start(
                            outp[:, ds((mc * 2 + lh) * 2048, 1024)],
                            o[:, ds(0, 1024)])
                        nc.scalar.activation(
                            o[:, ds(1024, 1024)], p3s[1][:], ident,
                            bias=b2c[:, ds(lh, 1)])
                        nc.sync.dma_start(
                            outp[:, ds((mc * 2 + lh) * 2048 + 1024, 1024)],
                            o[:, ds(1024, 1024)])

                    r0 = emit_relu(0)
                    emit_p2(1, 1)
                    emit_p3(0, 0, r0)
                    emit_p2(2, 1)
                    emit_p3(0, 1, r0)
                    for s in range(3, NS):
                        emit_p2(s, 1)
                    r1 = emit_relu(1)
                    emit_p3(1, 0, r1)
                    emit_p3(1, 1, r1)

    nc.compile()
    return nc


def kernel(x, a, W1, b1, W2, b2):
    global last_exec_time_ns, last_profile_json
    import ml_dtypes
    from concourse.bass_utils import run_bass_kernel_spmd

    if "nc" not in _cache:
        _cache["nc"] = _build()
    nc = _cache["nc"]

    x = np.asarray(x, np.float32)
    a = np.asarray(a, np.float32)
    W1 = np.asarray(W1, np.float32)
    b1 = np.asarray(b1, np.float32)
    W2 = np.asarray(W2, np.float32)
    b2 = np.asarray(b2, np.float32)

    # x part: [s, p<120, b*1024 + fh*512 + nl] = x[4g+b, 512s + nl, 120fh + p]
    xg = []
    for g in range(2):
        xpart = x[g * NB:(g + 1) * NB]                   # [4, 4096, 240]
        v = xpart.reshape(NB, NS, 512, 2, 120).transpose(1, 4, 3, 0, 2)
        xg.append(np.ascontiguousarray(v).reshape(NS, 120, 4096)
                  .astype(ml_dtypes.float8_e3m4))
    # a part (mc-major): aj[j][128k4 + p, 2048mc + 512q + c]
    #   = a[1024j + 512mc + c, 512k4 + 128q + p]
    aj = []
    for j in range(4):
        ajT = np.ascontiguousarray(a[j * NRC:(j + 1) * NRC, :].T)  # [4096,1024]
        v = ajT.reshape(8, 4, 128, 2, 512).transpose(0, 2, 3, 1, 4)
        aj.append(np.ascontiguousarray(v).reshape(1024, 4096)
                  .astype(ml_dtypes.float8_e3m4))
    # paired blocks per core variant
    xnbs = {}
    aths = {}
    for g in range(2):
        for j in range(4):
            blk = np.zeros((NS, 128, 6144), ml_dtypes.float8_e3m4)
            blk[:, 0:120, 0:4096] = xg[g]
            blk[:, :, 4096:6144] = aj[j].reshape(NS, 128, 4096)[:, :, 0:2048]
            xnbs[(g, j)] = np.ascontiguousarray(blk).reshape(NS * 128, 6144)
    for j in range(4):
        hi = aj[j].reshape(NS, 128, 4096)[:, :, 2048:4096]  # [8,128,2048]
        ath = hi.reshape(4, 2, 128, 2048).transpose(0, 2, 1, 3)
        aths[j] = np.ascontiguousarray(ath).reshape(512, 4096)
    cpkd = np.zeros((128, 436), np.float16)
    # w1: cpkd[p, fh*32 + h] = W1[120*fh + p, h]
    cpkd[0:120, 0:64] = W1.reshape(2, 120, 32).transpose(1, 0, 2) \
        .reshape(120, 64).astype(np.float16)
    cpkd[:, 64:192] = np.eye(128, dtype=np.float16)
    cpkd[:, 192:432] = np.tile(W2.astype(np.float16), (4, 1))
    cpkd[:, 432] = np.tile(b1, 4).astype(np.float16)
    cpkd[0:120, 433:435] = b2.reshape(2, 120).T.astype(np.float16)

    ins = []
    for c in range(8):
        g, j = c // 4, c % 4
        ins.append({"xnb": xnbs[(g, j)], "athd": aths[j], "cpkd": cpkd})

    trace = TRACE
    if trace:
        try:
            _install_ntff_hook()
        except Exception:
            trace = False
    r = run_bass_kernel_spmd(nc, ins, list(range(8)), trace=trace)
    last_exec_time_ns = r.exec_time_ns
    last_profile_json = r.profile_json

    res = np.empty((B, N, L), np.float32)
    for c in range(8):
        g, j = c // 4, c % 4
        # outp[p, mc, lh, b, c] -> out[4g+b, 1024j + 512mc + c, 120lh + p]
        arr = r.results[c]["outp"].reshape(120, 2, 2, NB, 512)
        res[g * NB:(g + 1) * NB, j * NRC:(j + 1) * NRC, :] = \
            arr.transpose(3, 1, 4, 2, 0).reshape(NB, NRC, L).astype(np.float32)
    return res
